# revision 25
# baseline (speedup 1.0000x reference)
"""Distributed Bass kernel for the quirky-softmax attention layer on 8 TRN2 NeuronCores.

Reference (N=4096, D=1024, fp32):
    Q = x@Wq + bq; K = x@Wk + bk; V = x@Wv + bv
    S = mask * (Q @ K.T)
    e = exp(S)
    out[i, j] = e[i, j] / rowsum(e)[j]       # quirky: denominator indexed by COLUMN
    return out @ V

Sharding: rows of x across 8 cores (512 rows each). Each core computes
K^T/Q^T/V for its shard in fp8 (e4m3) with power-of-2 prescales
(x*64, W*32, mask*2^-10, V*64, output *2^-18), all-gathers K^T and V,
computes e^T (scores transposed: j on partitions, local i on free axis),
local row-sums s[i] via a 2^-12-scaled ones-vector matmul (partition
reduction), all-gathers s, spreads it across partitions with PE
transposes, then out^T = V64 @ (e^T * 4096/denom). All big matmuls are
fp8 DoubleRow (2 k-tiles per instruction, 2x PE throughput).

All-gather bounce buffers use partition-major [128, ...] DRAM layouts so
every DMA moves 4KB-contiguous rows (128 fat descriptors per transfer).
"""

import os
import numpy as np
import ml_dtypes

N = 4096
D = 1024
NC = 8
R = N // NC      # 512 rows per core
P = 128
KT = D // P      # 8 contraction tiles (4 DoubleRow pairs)
MT = D // P      # 8 output-feature tiles
JT = N // P      # 32 j tiles
IT = R // P      # 4 i tiles

LAST_EXEC_NS = None
LAST_RES = None

_cache = {}


def _try_install_ntff_hook():
    """Best-effort registration of the axon NTFF profiling hook (for tracing)."""
    import sys, types

    if "antenv.axon_hooks" in sys.modules:
        return True
    try:
        from trn_agent_boot.trn_boot import _ntff_profile_via_ctypes

        hook = _ntff_profile_via_ctypes("/opt/axon/libaxon_pjrt.so")
        if hook is None:
            return False
        mod = types.ModuleType("antenv.axon_hooks")
        mod.get_axon_ntff_profile_hook = lambda: hook
        mod.set_axon_ntff_profile_hook = lambda h: None
        sys.modules["antenv.axon_hooks"] = mod
        import antenv

        antenv.axon_hooks = mod

        # zero-egress container: the artifact upload would block on network
        from concourse import bass_utils

        bass_utils.upload_artifacts = lambda tmpdir: tmpdir
        return True
    except Exception:
        return False


def _install_neff_cache():
    """Content-keyed NEFF cache: identical BIR -> skip the multi-minute walrus compile."""
    import hashlib
    import shutil

    from concourse import bass2jax, bass_utils

    if getattr(bass_utils, "_neff_cache_installed", False):
        return
    orig = bass_utils.compile_bir_kernel

    def cached(bir_json, tmpdir, neff_name="file.neff"):
        import re

        key = re.sub(rb'"line": \d+', b'"line": 0', bir_json)
        key += os.environ.get("BASS_LDW_OPT", "0").encode()
        h = hashlib.sha256(key).hexdigest()[:24]
        cdir = "/tmp/bass_neff_cache"
        os.makedirs(cdir, exist_ok=True)
        cpath = os.path.join(cdir, h + ".neff")
        if os.path.exists(cpath):
            dst = os.path.join(tmpdir, neff_name)
            shutil.copy(cpath, dst)
            return dst
        p = orig(bir_json, tmpdir, neff_name)
        try:
            shutil.copy(p, cpath)
        except OSError:
            pass
        return p

    bass_utils.compile_bir_kernel = cached
    bass2jax.compile_bir_kernel = cached
    bass_utils._neff_cache_installed = True

    if os.environ.get("BASS_LDW_OPT", "0") == "1":
        orig_run = bass_utils.run_command

        def run_ldw(cmd, *a, **kw):
            cmd = [
                c.replace("--enable-ldw-opt=false", "--enable-ldw-opt=true")
                if isinstance(c, str) else c
                for c in cmd
            ]
            return orig_run(cmd, *a, **kw)

        bass_utils.run_command = run_ldw


def _build():
    import concourse.bacc as bacc
    import concourse.mybir as mybir
    import concourse.tile as tile
    from concourse.masks import make_identity

    f32 = mybir.dt.float32
    bf16 = mybir.dt.bfloat16
    f8 = mybir.dt.float8e4
    DR = mybir.MatmulPerfMode.DoubleRow
    RG = [list(range(NC))]

    nc = bacc.Bacc("TRN2", target_bir_lowering=False, debug=False, num_devices=NC)

    # partition-major params: one fat DMA each
    x8 = nc.declare_dram_parameter("x8", [P, KT * R], f8, isOutput=False)
    mask8 = nc.declare_dram_parameter("mask8", [P, JT * R], bf16, isOutput=False)
    wq8 = nc.declare_dram_parameter("wq8", [P, KT * D], f8, isOutput=False)
    wk8 = nc.declare_dram_parameter("wk8", [P, KT * D], f8, isOutput=False)
    wv8 = nc.declare_dram_parameter("wv8", [P, KT * D], f8, isOutput=False)
    bq32 = nc.declare_dram_parameter("bq32", [P, MT], f32, isOutput=False)
    bk32 = nc.declare_dram_parameter("bk32", [P, MT], f32, isOutput=False)
    bvb64 = nc.declare_dram_parameter("bvb64", [P, D], f32, isOutput=False)
    outT = nc.declare_dram_parameter("outT", [D, R], f32, isOutput=True)

    def cdma(fn, dst, src, n):
        """Chunk a DMA along the partition dim into n parallel dma_starts:
        descriptors of one dma_start issue serially (~17GB/s), separate
        dma_starts run concurrently across rings."""
        pp = dst.shape[0]
        assert pp % n == 0 and src.shape[0] == pp
        step = pp // n
        for i in range(n):
            fn(dst[i * step:(i + 1) * step], src[i * step:(i + 1) * step])

    with tile.TileContext(nc) as tc:
        with tc.tile_pool(name="dram", bufs=1, space="DRAM") as dram, \
             tc.tile_pool(name="const", bufs=1) as const:
            kt_in = dram.tile([P, MT, R], f8)
            kt_ag = dram.tile([NC * P, MT, R], f8, addr_space="Shared")
            v_in = dram.tile([P, IT, D], f8)
            v_ag = dram.tile([NC * P, IT, D], f8, addr_space="Shared")
            s_in = dram.tile([1, R], f32)
            s_ag = dram.tile([NC, R], f32, addr_space="Shared")
            warm_in = dram.tile([1, 8], f32)
            warm_ag = dram.tile([NC, 8], f32, addr_space="Shared")

            # ---- CC engine warm-up: first collective pays ~11us setup +
            # core-skew barrier; burn it on a garbage-content 32B all-gather
            # with NO data dependencies so it triggers immediately ----
            nc.gpsimd.collective_compute(
                "AllGather", mybir.AluOpType.bypass, replica_groups=RG,
                ins=[warm_in.opt()], outs=[warm_ag.opt()],
            )

            # ---- resident inputs ----
            # DMA queues execute in order: scalar = x/weights (+ v bounce, vt,
            # outT), sync = kt bounce + kt slabs + s path + outT, gpsimd
            # (SWDGE) = mask bulk + odd kt slabs.
            xt8 = const.tile([P, KT, R], f8)
            wk_sb = const.tile([P, KT, D], f8)
            # first k-pair slices land first so the K projection starts early
            nc.scalar.dma_start(xt8[:, 0:2, :], x8.ap()[:, 0:2 * R])
            nc.sync.dma_start(wk_sb[:, 0:2, :], wk8.ap()[:, 0:2 * D])
            nc.scalar.dma_start(xt8[:, 2:KT, :], x8.ap()[:, 2 * R:KT * R])
            nc.sync.dma_start(wk_sb[:, 2:KT, :], wk8.ap()[:, 2 * D:KT * D])
            bk_sb = const.tile([P, MT], f32)
            nc.scalar.dma_start(bk_sb[:], bk32.ap())
            bq_sb = const.tile([P, MT], f32)
            nc.scalar.dma_start(bq_sb[:], bq32.ap())
            bv_sb = const.tile([P, D], f32)
            nc.scalar.dma_start(bv_sb[:], bvb64.ap())
            # mask bulk on sync after the wk chunks; done before the kt
            # slab stream needs the queue
            mask_sb = const.tile([P, JT, R], bf16)
            for q in range(4):
                nc.sync.dma_start(
                    mask_sb[:, q * 8:(q + 1) * 8, :],
                    mask8.ap()[:, q * 8 * R:(q + 1) * 8 * R],
                )
            # weight loads hoisted before the (data-gated) kt bounce so they
            # don't head-of-line block behind it on the scalar queue
            wq_sb = const.tile([P, KT, D], f8)
            nc.scalar.dma_start(wq_sb[:], wq8.ap())
            wv_sb = const.tile([P, KT, D], f8)
            nc.scalar.dma_start(wv_sb[:], wv8.ap())
            ones_sb = const.tile([P, 1], bf16)
            nc.vector.memset(ones_sb[:], 2.0 ** -12)
            ident8 = const.tile([8, 8], f32)
            make_identity(nc, ident8[:])

            qt8 = const.tile([P, KT, R], f8)
            kt_sb = const.tile([P, MT, R], f8)
            et_sb = const.tile([P, JT, R], bf16)
            e8_sb = const.tile([P, JT, R], f8)
            vt_all = const.tile([P, JT, D], f8)
            r_raw = const.tile([P, IT, NC], f32)
            s8_sb = const.tile([NC, R], f32)
            s_sb = const.tile([1, R], f32)

            # ---------------- projections (fp8 DoubleRow, kp-outer) ----------
            with tc.tile_pool(name="proj_sb", bufs=2) as proj_sb, \
                 tc.tile_pool(name="proj_ps", bufs=8, space="PSUM") as proj_ps:
                # K^T: lhsT = Wk pair-block, rhs = x^T pair. kp-outer so the
                # first 8 matmuls need only the first input chunk.
                psk = [proj_ps.tile([P, R], f32, tag="ps", name=f"ps_k{m}")
                       for m in range(MT)]
                for k in range(0, KT, 2):
                    for m in range(MT):
                        nc.tensor.matmul(
                            psk[m][:], wk_sb[:, k:k + 2, m * P:(m + 1) * P],
                            xt8[:, k:k + 2, :],
                            start=(k == 0), stop=(k == KT - 2), perf_mode=DR,
                        )
                for m in range(MT):
                    # 32*K^T = psum/64 + 32*bk -> fp8
                    nc.vector.tensor_scalar(
                        out=kt_sb[:, m, :], in0=psk[m][:],
                        scalar1=1.0 / 64.0, scalar2=bk_sb[:, m:m + 1],
                        op0=mybir.AluOpType.mult, op1=mybir.AluOpType.add,
                    )
                # bounce split across both HWDGE queues; AG triggered from
                # sync right behind its half so it fires the moment both land
                nc.scalar.dma_start(kt_in[0:64], kt_sb[0:64])
                nc.sync.dma_start(kt_in[64:128], kt_sb[64:128])
                nc.gpsimd.collective_compute(
                    "AllGather", mybir.AluOpType.bypass, replica_groups=RG,
                    ins=[kt_in.opt()], outs=[kt_ag.opt()],
                )

                # Q^T
                psq = [proj_ps.tile([P, R], f32, tag="ps", name=f"ps_q{m}")
                       for m in range(MT)]
                for k in range(0, KT, 2):
                    for m in range(MT):
                        nc.tensor.matmul(
                            psq[m][:], wq_sb[:, k:k + 2, m * P:(m + 1) * P],
                            xt8[:, k:k + 2, :],
                            start=(k == 0), stop=(k == KT - 2), perf_mode=DR,
                        )
                for m in range(MT):
                    nc.vector.tensor_scalar(
                        out=qt8[:, m, :], in0=psq[m][:],
                        scalar1=1.0 / 64.0, scalar2=bq_sb[:, m:m + 1],
                        op0=mybir.AluOpType.mult, op1=mybir.AluOpType.add,
                    )

                # V (natural layout): lhsT = x^T pair, rhs = Wv pair
                v_sb = proj_sb.tile([P, IT, D], f8, tag="pout")
                psv = [proj_ps.tile([P, 512], f32, tag="ps", name=f"ps_v{v}")
                       for v in range(8)]
                for k in range(0, KT, 2):
                    for it in range(IT):
                        for c2 in range(2):
                            nc.tensor.matmul(
                                psv[it * 2 + c2][:],
                                xt8[:, k:k + 2, it * P:(it + 1) * P],
                                wv_sb[:, k:k + 2, c2 * 512:(c2 + 1) * 512],
                                start=(k == 0), stop=(k == KT - 2), perf_mode=DR,
                            )
                for it in range(IT):
                    for c2 in range(2):
                        # 64*V = psum/32 + 64*bv  (bias varies along free axis)
                        vtmp = proj_sb.tile([P, 512], bf16, tag="vtmp",
                                            name=f"vtmp{it}_{c2}")
                        nc.scalar.activation(
                            vtmp[:], psv[it * 2 + c2][:],
                            mybir.ActivationFunctionType.Copy,
                            bias=0.0, scale=1.0 / 32.0,
                        )
                        nc.vector.tensor_add(
                            v_sb[:, it, c2 * 512:(c2 + 1) * 512], vtmp[:],
                            bv_sb[:, c2 * 512:(c2 + 1) * 512],
                        )
                nc.scalar.dma_start(v_in[:], v_sb[:])
                nc.gpsimd.collective_compute(
                    "AllGather", mybir.AluOpType.bypass, replica_groups=RG,
                    ins=[v_in.opt()], outs=[v_ag.opt()],
                )
                # prefetch all V slabs for the AV phase (scalar queue is idle
                # by the time the collective lands)
                for g in range(NC):
                    nc.scalar.dma_start(
                        vt_all[:, g * IT:(g + 1) * IT, :],
                        v_ag[g * P:(g + 1) * P, :, :],
                    )

            # ---------------- scores^T + exp + rowsums ----------------
            with tc.tile_pool(name="ktp", bufs=4) as ktp, \
                 tc.tile_pool(name="tp", bufs=4) as tp, \
                 tc.tile_pool(name="sc_ps", bufs=4, space="PSUM") as sc_ps, \
                 tc.tile_pool(name="s1_ps", bufs=1, space="PSUM") as s1_ps, \
                 tc.tile_pool(name="tr_ps", bufs=1, space="PSUM") as tr_ps:
                s1 = s1_ps.tile([1, R], f32)

                def rowsum_mm(idx):
                    nc.tensor.matmul(
                        s1[:], ones_sb[:], et_sb[:, idx, :],
                        start=(idx == 0), stop=(idx == JT - 1),
                    )

                ktc = None
                for t in range(JT):
                    c, jj = t // IT, t % IT
                    if jj == 0:
                        ktc = ktp.tile([P, MT, R], f8, tag="kt", name=f"ktc{c}")
                        nc.sync.dma_start(ktc[:], kt_ag[c * P:(c + 1) * P, :, :])
                    ps = sc_ps.tile([P, R], f32, tag="ps", name=f"ps_s{t}")
                    for k in range(0, KT, 2):
                        nc.tensor.matmul(
                            ps[:], ktc[:, k:k + 2, jj * P:(jj + 1) * P],
                            qt8[:, k:k + 2, :],
                            start=(k == 0), stop=(k == KT - 2), perf_mode=DR,
                        )
                    # lag the rowsum matmul so PE never waits on ACT
                    if t >= 2:
                        rowsum_mm(t - 2)
                    msked = tp.tile([P, R], bf16, tag="msked", name=f"msk{t}")
                    nc.vector.tensor_mul(msked[:], ps[:], mask_sb[:, t, :])
                    nc.scalar.activation(
                        et_sb[:, t, :], msked[:], mybir.ActivationFunctionType.Exp
                    )
                rowsum_mm(JT - 2)
                rowsum_mm(JT - 1)

                # s1 = denom(local rows) * 2^-12 ; all-gather, then spread the
                # [8, 512] result across partitions with PE transposes
                nc.vector.tensor_copy(s_sb[:], s1[:])
                nc.sync.dma_start(s_in[:], s_sb[:])
                nc.gpsimd.collective_compute(
                    "AllGather", mybir.AluOpType.bypass, replica_groups=RG,
                    ins=[s_in.opt()], outs=[s_ag.opt()],
                )
                nc.sync.dma_start(s8_sb[:], s_ag[:, :])
                trp = tr_ps.tile([P, IT, NC], f32)
                for tl in range(IT):
                    nc.tensor.transpose(
                        trp[:, tl, :], s8_sb[0:NC, tl * P:(tl + 1) * P], ident8[:]
                    )
                # r = 4096/denom (the 2^-12 prescale of ones makes this ~1.0)
                nc.vector.reciprocal(r_raw[:], trp[:])
                # e8 = e * r on the ACT engine (DVE is the busier engine)
                for t in range(JT):
                    nc.scalar.activation(
                        e8_sb[:, t, :], et_sb[:, t, :],
                        mybir.ActivationFunctionType.Copy,
                        bias=0.0, scale=r_raw[:, t % IT, t // IT:t // IT + 1],
                    )

            # ---------------- out^T = V64 @ e8 (j-contraction, DoubleRow) ----
            with tc.tile_pool(name="op", bufs=1) as op, \
                 tc.tile_pool(name="out_ps", bufs=1, space="PSUM") as out_ps:
                pso = [out_ps.tile([P, R], f32, name=f"pso{m}") for m in range(MT)]
                for t in range(0, JT, 2):
                    for m in range(MT):
                        nc.tensor.matmul(
                            pso[m][:], vt_all[:, t:t + 2, m * P:(m + 1) * P],
                            e8_sb[:, t:t + 2, :],
                            start=(t == 0), stop=(t == JT - 2), perf_mode=DR,
                        )
                # per-m scale + store: each m's result drains while the PE is
                # still finishing later m's of the last contraction pair
                ot_sb = op.tile([P, MT, R], f32)
                outT_pm = outT.ap().rearrange("(m p) i -> p m i", p=P)
                for m in range(MT):
                    nc.vector.tensor_scalar_mul(
                        ot_sb[:, m, :], pso[m][:], 2.0 ** -18
                    )
                    odma = nc.sync.dma_start if m % 2 == 0 else nc.scalar.dma_start
                    odma(outT_pm[:, m:m + 1, :], ot_sb[:, m:m + 1, :])

    nc.finalize()
    return nc


def _get_nc():
    if "nc" not in _cache:
        _cache["nc"] = _build()
    return _cache["nc"]


def kernel(x, mask, Wq, bq, Wk, bk, Wv, bv):
    global LAST_EXEC_NS
    _install_neff_cache()
    from concourse.bass_utils import run_bass_kernel_spmd

    f8 = ml_dtypes.float8_e4m3fn
    bf = ml_dtypes.bfloat16
    x = np.asarray(x, dtype=np.float32)
    mask = np.asarray(mask, dtype=np.float32)

    def pmajor(a, nt):
        # [nt*128, F] -> [128, nt*F] partition-major
        F = a.shape[1]
        return np.ascontiguousarray(
            a.reshape(nt, P, F).transpose(1, 0, 2).reshape(P, nt * F)
        )

    wq_8 = pmajor(np.asarray(Wq, dtype=np.float32) * 32, KT).astype(f8)
    wk_8 = pmajor(np.asarray(Wk, dtype=np.float32) * 32, KT).astype(f8)
    wv_8 = pmajor(np.asarray(Wv, dtype=np.float32) * 32, KT).astype(f8)
    bq_32 = np.ascontiguousarray((np.asarray(bq, np.float32) * 32).reshape(MT, P).T)
    bk_32 = np.ascontiguousarray((np.asarray(bk, np.float32) * 32).reshape(MT, P).T)
    bvb = np.ascontiguousarray(
        np.broadcast_to(np.asarray(bv, dtype=np.float32) * 64, (P, D))
    )

    in_maps = []
    for c in range(NC):
        rows = slice(c * R, (c + 1) * R)
        xT = np.ascontiguousarray(x[rows, :].T) * 64
        maskT = np.ascontiguousarray(mask[rows, :].T) * 2.0 ** -10
        in_maps.append({
            "x8": pmajor(xT, KT).astype(f8),
            "mask8": pmajor(maskT, JT).astype(bf),
            "wq8": wq_8, "wk8": wk_8, "wv8": wv_8,
            "bq32": bq_32, "bk32": bk_32, "bvb64": bvb,
        })

    nc = _get_nc()
    trace = os.environ.get("BASS_KERNEL_TRACE", "0") == "1"
    if trace:
        trace = _try_install_ntff_hook()
    res = run_bass_kernel_spmd(
        nc, in_maps, core_ids=list(range(NC)), trace=trace,
        **({"trace_cores": [0]} if trace else {}),
    )
    LAST_EXEC_NS = res.exec_time_ns
    globals()["LAST_RES"] = res
    out = np.concatenate(
        [res.results[c]["outT"].T for c in range(NC)], axis=0
    ).astype(np.float32)
    return out
